# revision 5
# baseline (speedup 1.0000x reference)
"""Trainium2 Bass kernel v2: entmax-1.5 along last dim of x[8,16,1024,1024] f32.

Shards rows data-parallel across 8 NeuronCores (full input in, full output
out). Per-row algorithm (d=1024), validated in numpy against the float64
sort-based reference on 32k real rows (max |y-yref| = 3.2e-3, gate 2e-2):

  h = f16(x/2)  (tau iterated in h-space; y = relu(h - tau)^2)
  seed: closed-form k=8 threshold over the 8 chunk maxima of h
        (max of each 128-col chunk) -- a from-below estimate of tau.
  2x closed-form support iterations, each with the threshold QUANTIZED to
  f16 and used consistently (scalar of max/is_gt, the -1024*t correction,
  and the ACT bias), so off-support elements contribute exactly zero:
    k  = sum(h > t16)            DVE ts is_gt + accum      (f16 4x)
    r  = max(h, t16), A1 = sum r DVE ts max   + accum      (f16 4x)
    s2 = sum (r - t16)^2         ACT Square(r, bias=-t16) + accum
    s1 = A1 - 1024*t16;  t += (s1 - sqrt(s1^2 - k*(s2-1)))/k
  final: r3 = max(h, t16);  y = Square(r3 - t16) -> f32, DMA out.

HW facts this schedule is built on (measured on these cores):
  - GPSIMD streaming is ~20x slower than the cost model (29.5us per
    [128,1024] op) -> Pool is used for NOTHING.
  - DMA is fast: full 64 MiB in + 64 MiB out measured at ~226 us/core.
  - DVE f16 tensor_scalar runs 4x (327ns), f32 2x (594ns); ACT ~1us/pass.
  - ACT Square+Sqrt live in one LUT table-set (no swap cost).

Engine budget per [128,1024] tile: DVE ~3.3us (cast 549, chunkmax 1127,
2x(k 327 + r 327), final r 327), ACT ~2.9us (2x sq 930 + final y 930),
DMA ~1.8-2.8us. Update chains are [P,16] tinies on DVE/ACT.
"""

import sys

sys.path.insert(0, "/opt/trn_rl_repo")
sys.path.insert(0, "/opt/trn_rl_repo/concourse")

from contextlib import ExitStack

import numpy as np

D = 1024
P = 128
N_CORES = 8


def build_program(n_rows, group_tiles=16, dma_batch=4, debug=False, reps=1,
                  xp_bufs=4, hp_mult=2, tr_bufs=4, yp_bufs=2, smp_bufs=3,
                  cast_act_mod=0, ablate=()):
    """cast_act_mod: every k-th cast batch goes to ACT instead of DVE
    (0 = all casts on DVE)."""
    import concourse.bacc as bacc
    import concourse.tile as tile
    from concourse import mybir

    F32 = mybir.dt.float32
    F16 = mybir.dt.float16
    ALU = mybir.AluOpType
    ACTF = mybir.ActivationFunctionType
    AX = mybir.AxisListType

    G = group_tiles
    B = dma_batch
    nb = G // B
    T = n_rows // P
    assert n_rows % P == 0 and T % G == 0 and G % B == 0
    n_groups = T // G

    nc = bacc.Bacc(
        "TRN2", target_bir_lowering=False, debug=debug, enable_asserts=False
    )
    x = nc.dram_tensor("x", [n_rows, D], F32, kind="ExternalInput").ap()
    y = nc.dram_tensor("y", [n_rows, D], F32, kind="ExternalOutput").ap()

    with tile.TileContext(nc) as tc, ExitStack() as ctx:
        xp = ctx.enter_context(tc.tile_pool(name="xp", bufs=xp_bufs))
        hp = ctx.enter_context(tc.tile_pool(name="hp", bufs=hp_mult * nb))
        c8p = ctx.enter_context(tc.tile_pool(name="c8p", bufs=2))
        rt = ctx.enter_context(tc.tile_pool(name="rt", bufs=tr_bufs))
        kt = ctx.enter_context(tc.tile_pool(name="kt", bufs=tr_bufs))
        sqt = ctx.enter_context(tc.tile_pool(name="sqt", bufs=tr_bufs))
        yp = ctx.enter_context(tc.tile_pool(name="yp", bufs=yp_bufs))
        smp = ctx.enter_context(tc.tile_pool(name="smp", bufs=smp_bufs))

        state = {}
        src = {"t": x}
        cast_ctr = {"i": 0}

        def hview(g, j):
            """[P, 8, 128] f16 view of tile j of group g."""
            hb = state[g]["hb"][j // B]
            return hb[:, j % B]

        def stage_load_cast_seed(g):
            g_row0 = g * G * P
            hb = []
            c8 = c8p.tile([P, G, 8], F32, tag="c8")
            for b in range(nb):
                xt = xp.tile([P, B, 8, 128], F32, tag="x")
                r0 = g_row0 + b * B * P
                if "no_dma_in" not in ablate:
                    nc.sync.dma_start(
                        xt[:],
                        src["t"][r0 : r0 + B * P, :].rearrange(
                            "(a p) (c e) -> p a c e", p=P, e=128
                        ),
                    )
                else:
                    nc.vector.memset(xt[:, 0:1, 0:8], 0.5)
                ht = hp.tile([P, B, 8, 128], F16, tag="h")
                cast_ctr["i"] += 1
                if cast_act_mod and cast_ctr["i"] % cast_act_mod == 0:
                    nc.scalar.activation(ht[:], xt[:], ACTF.Copy, scale=0.5)
                else:
                    nc.vector.tensor_scalar(
                        ht[:], xt[:], 0.5, None, op0=ALU.mult
                    )
                hb.append(ht)
                for jj in range(B):
                    j = b * B + jj
                    nc.vector.tensor_reduce(
                        c8[:, j, :],
                        ht[:, jj],
                        axis=AX.X,
                        op=ALU.max,
                    )
            state[g] = {"hb": hb}

            # seed chain: k=8 closed form over c8 (all [P,G] f32 tinies)
            m = smp.tile([P, G], F32, tag="m")
            nc.vector.tensor_reduce(m[:], c8[:], axis=AX.X, op=ALU.max)
            S = smp.tile([P, G], F32, tag="S")
            nc.vector.tensor_reduce(S[:], c8[:], axis=AX.X, op=ALU.add)
            c8sq = c8p.tile([P, G, 8], F32, tag="c8sq")
            nc.vector.tensor_tensor(c8sq[:], c8[:], c8[:], op=ALU.mult)
            Q = smp.tile([P, G], F32, tag="Q")
            nc.vector.tensor_reduce(Q[:], c8sq[:], axis=AX.X, op=ALU.add)
            # s1_8 = S - 8m ; s2_8 = Q - 2mS + 8m^2
            mm = smp.tile([P, G], F32, tag="mm")
            nc.vector.tensor_scalar_mul(mm[:], m[:], -8.0)
            s1 = smp.tile([P, G], F32, tag="s1")
            nc.vector.tensor_tensor(s1[:], S[:], mm[:], op=ALU.add)
            mS = smp.tile([P, G], F32, tag="mS")
            nc.vector.tensor_tensor(mS[:], m[:], S[:], op=ALU.mult)
            m2 = smp.tile([P, G], F32, tag="m2")
            nc.vector.tensor_tensor(m2[:], m[:], m[:], op=ALU.mult)
            a1 = smp.tile([P, G], F32, tag="a1")
            nc.vector.tensor_scalar_mul(a1[:], mS[:], -2.0)
            a2 = smp.tile([P, G], F32, tag="a2")
            nc.vector.tensor_scalar_mul(a2[:], m2[:], 8.0)
            s2a = smp.tile([P, G], F32, tag="s2a")
            nc.vector.tensor_tensor(s2a[:], Q[:], a1[:], op=ALU.add)
            s2 = smp.tile([P, G], F32, tag="s2")
            nc.vector.tensor_tensor(s2[:], s2a[:], a2[:], op=ALU.add)
            # disc = s1^2 - 8*s2 + 8 ; t0 = m + (s1 - sqrt(disc))/8
            q = smp.tile([P, G], F32, tag="q")
            nc.vector.tensor_tensor(q[:], s1[:], s1[:], op=ALU.mult)
            b1 = smp.tile([P, G], F32, tag="b1")
            nc.vector.tensor_scalar(
                b1[:], s2[:], -8.0, 8.0, op0=ALU.mult, op1=ALU.add
            )
            d0 = smp.tile([P, G], F32, tag="d0")
            nc.vector.tensor_tensor(d0[:], q[:], b1[:], op=ALU.add)
            dn = smp.tile([P, G], F32, tag="dn")
            nc.vector.tensor_scalar_max(dn[:], d0[:], 1e-30)
            root = smp.tile([P, G], F32, tag="root")
            nc.scalar.activation(root[:], dn[:], ACTF.Sqrt)
            num = smp.tile([P, G], F32, tag="num")
            nc.vector.tensor_tensor(num[:], s1[:], root[:], op=ALU.subtract)
            th = smp.tile([P, G], F32, tag="th")
            nc.vector.tensor_scalar_mul(th[:], num[:], 0.125)
            t0 = smp.tile([P, G], F32, tag="t0")
            nc.vector.tensor_tensor(t0[:], m[:], th[:], op=ALU.add)
            state[g]["t"] = t0

        def q16(g):
            """Quantize t to f16; return (t16f [f32 holding f16 values],
            tb = -t16f) for use as consistent scalars/bias."""
            t_cur = state[g]["t"]
            t16h = smp.tile([P, G], F16, tag="t16h")
            nc.vector.tensor_scalar_mul(t16h[:], t_cur[:], 1.0)
            t16f = smp.tile([P, G], F32, tag="t16f")
            nc.vector.tensor_scalar_mul(t16f[:], t16h[:], 1.0)
            tb = smp.tile([P, G], F32, tag="tb")
            nc.vector.tensor_scalar_mul(tb[:], t16f[:], -1.0)
            return t16f, tb

        def stage_iter(g, it):
            t16f, tb = q16(g)
            K = smp.tile([P, G], F32, tag="K")
            A1 = smp.tile([P, G], F32, tag="A1")
            S2 = smp.tile([P, G], F32, tag="S2")
            for j in range(G):
                h_j = hview(g, j)
                t_col = t16f[:, j : j + 1]
                kct = kt.tile([P, 8, 128], F16, tag="k")
                nc.vector.tensor_scalar(
                    kct[:], h_j, t_col, None,
                    op0=ALU.is_gt, op1=ALU.add,
                    accum_out=K[:, j : j + 1],
                )
                rte = rt.tile([P, 8, 128], F16, tag="r")
                nc.vector.tensor_scalar(
                    rte[:], h_j, t_col, None,
                    op0=ALU.max, op1=ALU.add,
                    accum_out=A1[:, j : j + 1],
                )
                sqe = sqt.tile([P, 8, 128], F16, tag="sq")
                nc.scalar.activation(
                    sqe[:], rte[:], ACTF.Square,
                    scale=1.0, bias=tb[:, j : j + 1],
                    accum_out=S2[:, j : j + 1],
                )
            # update: s1 = A1 - 1024*t16; t += (s1 - sqrt(s1^2 - k(s2-1)))/k
            tm = smp.tile([P, G], F32, tag="tm")
            nc.vector.tensor_scalar_mul(tm[:], t16f[:], -1024.0)
            s1 = smp.tile([P, G], F32, tag="s1i")
            nc.vector.tensor_tensor(s1[:], A1[:], tm[:], op=ALU.add)
            s1g = smp.tile([P, G], F32, tag="s1g")
            nc.vector.tensor_scalar_max(s1g[:], s1[:], 1e-6)
            g1 = smp.tile([P, G], F32, tag="g1")
            nc.vector.tensor_scalar(g1[:], S2[:], -1.0, None, op0=ALU.add)
            kd = smp.tile([P, G], F32, tag="kd")
            nc.vector.tensor_tensor(kd[:], g1[:], K[:], op=ALU.mult)
            q = smp.tile([P, G], F32, tag="qi")
            nc.vector.tensor_tensor(q[:], s1g[:], s1g[:], op=ALU.mult)
            di = smp.tile([P, G], F32, tag="di")
            nc.vector.tensor_tensor(di[:], q[:], kd[:], op=ALU.subtract)
            dn = smp.tile([P, G], F32, tag="dni")
            nc.vector.tensor_scalar_max(dn[:], di[:], 1e-30)
            root = smp.tile([P, G], F32, tag="rooti")
            nc.scalar.activation(root[:], dn[:], ACTF.Sqrt)
            num = smp.tile([P, G], F32, tag="numi")
            nc.vector.tensor_tensor(num[:], s1g[:], root[:], op=ALU.subtract)
            kg = smp.tile([P, G], F32, tag="kg")
            nc.vector.tensor_scalar_max(kg[:], K[:], 1.0)
            kinv = smp.tile([P, G], F32, tag="kinv")
            nc.vector.reciprocal(kinv[:], kg[:])
            pr = smp.tile([P, G], F32, tag="pr")
            nc.vector.tensor_tensor(pr[:], num[:], kinv[:], op=ALU.mult)
            t_new = smp.tile([P, G], F32, tag="tn")
            nc.vector.tensor_tensor(
                t_new[:], state[g]["t"][:], pr[:], op=ALU.add
            )
            state[g]["t"] = t_new

        def stage_final(g):
            g_row0 = g * G * P
            t16f, tb = q16(g)
            for b in range(nb):
                yt = yp.tile([P, B, 8, 128], F32, tag="y")
                for jj in range(B):
                    j = b * B + jj
                    h_j = hview(g, j)
                    rte = rt.tile([P, 8, 128], F16, tag="r")
                    nc.vector.tensor_scalar(
                        rte[:], h_j, t16f[:, j : j + 1], None, op0=ALU.max
                    )
                    nc.scalar.activation(
                        yt[:, jj], rte[:], ACTF.Square,
                        scale=1.0, bias=tb[:, j : j + 1],
                    )
                r0 = g_row0 + b * B * P
                if "no_dma_out" not in ablate:
                    nc.sync.dma_start(
                        y[r0 : r0 + B * P, :].rearrange(
                            "(a p) (c e) -> p a c e", p=P, e=128
                        ),
                        yt[:],
                    )

        for rep in range(reps):
            if rep == 1:
                src["t"] = y
            for p0 in range(0, n_groups, 2):
                pair = [g for g in (p0, p0 + 1) if g < n_groups]
                for g in pair:
                    stage_load_cast_seed(g)
                for it in range(2):
                    for g in pair:
                        stage_iter(g, it)
                for g in pair:
                    stage_final(g)

    nc.compile()
    return nc


_PROGRAM = None
_PROGRAM_ROWS = None


def _get_program(rows_per_core):
    global _PROGRAM, _PROGRAM_ROWS
    if _PROGRAM is None or _PROGRAM_ROWS != rows_per_core:
        _PROGRAM = build_program(rows_per_core)
        _PROGRAM_ROWS = rows_per_core
    return _PROGRAM


def run_sharded(flat_x, trace=False):
    from concourse.bass_utils import run_bass_kernel_spmd

    n_rows = flat_x.shape[0]
    rows_per = n_rows // N_CORES
    assert rows_per * N_CORES == n_rows
    nc = _get_program(rows_per)
    in_maps = [
        {"x": np.ascontiguousarray(flat_x[i * rows_per : (i + 1) * rows_per])}
        for i in range(N_CORES)
    ]
    res = run_bass_kernel_spmd(nc, in_maps, list(range(N_CORES)), trace=trace)
    y = np.concatenate([res.results[i]["y"] for i in range(N_CORES)], axis=0)
    return y, res


def kernel(x):
    x = np.ascontiguousarray(np.asarray(x), dtype=np.float32)
    orig_shape = x.shape
    flat = x.reshape(-1, D)
    y, _ = run_sharded(flat)
    return y.reshape(orig_shape)


# revision 6
# speedup vs baseline: 7.2853x; 7.2853x over previous
"""Trainium2 Bass kernel v3: entmax-1.5 along last dim of x[8,16,1024,1024] f32.

Row-parallel over 8 NeuronCores. Validated in numpy against the float64
sort reference on 32k real rows: max |y-yref| = 2.55e-3 (gate 2e-2).

Algorithm per row (d=1024), h = f16(x/2), tau in h-space:
  seed: closed-form k=8 threshold over the 8 chunk maxima of h (ACT Sqrt).
  2 support iterations with threshold quantized to f16 and used
  consistently (off-support terms cancel exactly):
    k  = sum(h > t16)                  DVE is_gt + accum
    r  = max(h, t16), A1 = sum r       DVE max + accum
    s2 = sum (r - t16)^2               ACT Square(r, bias=-t16) + accum
    s1 = A1 - 1024 t16; theta = 2-step-Newton root of
         k th^2 - 2 s1 th + (s2-1), clamped to [-2, s1/k]  (all-DVE chain)
  final: r3 = max(h, t16); y = Square(r3 - t16) -> f32.

HW lessons encoded here (all measured on these cores):
  - GPSIMD streaming is ~20x slower than modeled -> Pool unused.
  - 4-dim DMA access patterns (512B descriptors) degrade badly with 8 cores
    running concurrently -> v1-style "(a p) m -> p a m" 4KB descriptors.
  - accum_out ops run ~1us (not 327ns); ACT Square with bias-AP+accum
    ~2.6us; plain streaming ops are fast.
  - Cross-engine dependency hops in the t-update chains are expensive ->
    chains are all-DVE (2-step Newton instead of sqrt; ACT Sqrt only in
    the per-group seed chain), and the two groups of an emission pair
    share one [P, 2G] chain.
"""

import sys

sys.path.insert(0, "/opt/trn_rl_repo")
sys.path.insert(0, "/opt/trn_rl_repo/concourse")

from contextlib import ExitStack

import numpy as np

D = 1024
P = 128
N_CORES = 8


def build_program(n_rows, group_tiles=16, dma_batch=4, debug=False, reps=1,
                  xp_bufs=4, hp_mult=2, tr_bufs=4, yp_bufs=2, smp_bufs=3,
                  ablate=()):
    import concourse.bacc as bacc
    import concourse.tile as tile
    from concourse import mybir

    F32 = mybir.dt.float32
    F16 = mybir.dt.float16
    ALU = mybir.AluOpType
    ACTF = mybir.ActivationFunctionType
    AX = mybir.AxisListType

    G = group_tiles
    B = dma_batch
    nb = G // B
    T = n_rows // P
    assert n_rows % P == 0 and T % G == 0 and G % B == 0
    n_groups = T // G

    nc = bacc.Bacc(
        "TRN2", target_bir_lowering=False, debug=debug, enable_asserts=False
    )
    x = nc.dram_tensor("x", [n_rows, D], F32, kind="ExternalInput").ap()
    y = nc.dram_tensor("y", [n_rows, D], F32, kind="ExternalOutput").ap()

    with tile.TileContext(nc) as tc, ExitStack() as ctx:
        xp = ctx.enter_context(tc.tile_pool(name="xp", bufs=xp_bufs))
        hp = ctx.enter_context(tc.tile_pool(name="hp", bufs=hp_mult * nb))
        c8p = ctx.enter_context(tc.tile_pool(name="c8p", bufs=2))
        rt = ctx.enter_context(tc.tile_pool(name="rt", bufs=tr_bufs))
        kt = ctx.enter_context(tc.tile_pool(name="kt", bufs=tr_bufs))
        sqt = ctx.enter_context(tc.tile_pool(name="sqt", bufs=tr_bufs))
        yp = ctx.enter_context(tc.tile_pool(name="yp", bufs=yp_bufs))
        smp = ctx.enter_context(tc.tile_pool(name="smp", bufs=smp_bufs))

        state = {}
        src = {"t": x}

        # pair-level state: W = columns of the merged chain (2G normally)
        def hview(pair_state, q):
            """flat [P, 1024] f16 view of pair-tile q (0..W-1)."""
            hb = pair_state["hb"][q // B]
            return hb[:, q % B, :]

        def stage_load_cast_seed(pair):
            """Load, cast, chunk-max and seed for ALL groups of the pair;
            one merged [P, W] chain (W = 16*len(pair))."""
            W = G * len(pair)
            ps = {"hb": [], "W": W, "pair": pair}
            c8 = c8p.tile([P, W, 8], F32, tag="c8")
            for gi, g in enumerate(pair):
                g_row0 = g * G * P
                for b in range(nb):
                    xt = xp.tile([P, B, D], F32, tag="x")
                    r0 = g_row0 + b * B * P
                    if "no_dma_in" not in ablate:
                        nc.sync.dma_start(
                            xt[:],
                            src["t"][r0 : r0 + B * P, :].rearrange(
                                "(a p) m -> p a m", p=P
                            ),
                        )
                    else:
                        nc.vector.memset(xt[:, 0:1, 0:8], 0.5)
                    ht = hp.tile([P, B, D], F16, tag="h")
                    nc.vector.tensor_scalar(
                        ht[:], xt[:], 0.5, None, op0=ALU.mult
                    )
                    ps["hb"].append(ht)
                    for jj in range(B):
                        q = gi * G + b * B + jj
                        nc.vector.tensor_reduce(
                            c8[:, q, :],
                            ht[:, jj, :].rearrange("p (c e) -> p c e", e=128),
                            axis=AX.X,
                            op=ALU.max,
                        )

            # merged seed chain (k=8 closed form, ACT Sqrt for the root)
            m = smp.tile([P, W], F32, tag="m")
            nc.vector.tensor_reduce(m[:], c8[:], axis=AX.X, op=ALU.max)
            S = smp.tile([P, W], F32, tag="S")
            nc.vector.tensor_reduce(S[:], c8[:], axis=AX.X, op=ALU.add)
            c8sq = c8p.tile([P, W, 8], F32, tag="c8sq")
            nc.vector.tensor_tensor(c8sq[:], c8[:], c8[:], op=ALU.mult)
            Q = smp.tile([P, W], F32, tag="Q")
            nc.vector.tensor_reduce(Q[:], c8sq[:], axis=AX.X, op=ALU.add)
            mm = smp.tile([P, W], F32, tag="mm")
            nc.vector.tensor_scalar_mul(mm[:], m[:], -8.0)
            s1 = smp.tile([P, W], F32, tag="s1")
            nc.vector.tensor_tensor(s1[:], S[:], mm[:], op=ALU.add)
            mS = smp.tile([P, W], F32, tag="mS")
            nc.vector.tensor_tensor(mS[:], m[:], S[:], op=ALU.mult)
            m2 = smp.tile([P, W], F32, tag="m2")
            nc.vector.tensor_tensor(m2[:], m[:], m[:], op=ALU.mult)
            a1 = smp.tile([P, W], F32, tag="a1")
            nc.vector.tensor_scalar_mul(a1[:], mS[:], -2.0)
            a2 = smp.tile([P, W], F32, tag="a2")
            nc.vector.tensor_scalar_mul(a2[:], m2[:], 8.0)
            s2a = smp.tile([P, W], F32, tag="s2a")
            nc.vector.tensor_tensor(s2a[:], Q[:], a1[:], op=ALU.add)
            s2 = smp.tile([P, W], F32, tag="s2")
            nc.vector.tensor_tensor(s2[:], s2a[:], a2[:], op=ALU.add)
            q_ = smp.tile([P, W], F32, tag="q")
            nc.vector.tensor_tensor(q_[:], s1[:], s1[:], op=ALU.mult)
            b1 = smp.tile([P, W], F32, tag="b1")
            nc.vector.tensor_scalar(
                b1[:], s2[:], -8.0, 8.0, op0=ALU.mult, op1=ALU.add
            )
            d0 = smp.tile([P, W], F32, tag="d0")
            nc.vector.tensor_tensor(d0[:], q_[:], b1[:], op=ALU.add)
            dn = smp.tile([P, W], F32, tag="dn")
            nc.vector.tensor_scalar_max(dn[:], d0[:], 1e-30)
            root = smp.tile([P, W], F32, tag="root")
            nc.scalar.activation(root[:], dn[:], ACTF.Sqrt)
            num = smp.tile([P, W], F32, tag="num")
            nc.vector.tensor_tensor(num[:], s1[:], root[:], op=ALU.subtract)
            th = smp.tile([P, W], F32, tag="th")
            nc.vector.tensor_scalar_mul(th[:], num[:], 0.125)
            t0 = smp.tile([P, W], F32, tag="t0")
            nc.vector.tensor_tensor(t0[:], m[:], th[:], op=ALU.add)
            ps["t"] = t0
            return ps

        def q16(ps):
            W = ps["W"]
            if "fixed_t" in ablate:
                t16f = smp.tile([P, W], F32, tag="t16f")
                nc.vector.memset(t16f[:], 1.25)
                tb = smp.tile([P, W], F32, tag="tb")
                nc.vector.memset(tb[:], -1.25)
                return t16f, tb
            t16h = smp.tile([P, W], F16, tag="t16h")
            nc.vector.tensor_scalar_mul(t16h[:], ps["t"][:], 1.0)
            t16f = smp.tile([P, W], F32, tag="t16f")
            nc.vector.tensor_scalar_mul(t16f[:], t16h[:], 1.0)
            tb = smp.tile([P, W], F32, tag="tb")
            nc.vector.tensor_scalar_mul(tb[:], t16f[:], -1.0)
            return t16f, tb

        def stage_iter(ps, it):
            W = ps["W"]
            t16f, tb = q16(ps)
            K = smp.tile([P, W], F32, tag="K")
            A1 = smp.tile([P, W], F32, tag="A1")
            S2 = smp.tile([P, W], F32, tag="S2")
            for q in range(W):
                h_q = hview(ps, q)
                t_col = t16f[:, q : q + 1]
                kct = kt.tile([P, D], F16, tag="k")
                nc.vector.tensor_scalar(
                    kct[:], h_q, t_col, None,
                    op0=ALU.is_gt, op1=ALU.add,
                    accum_out=K[:, q : q + 1],
                )
                rte = rt.tile([P, D], F16, tag="r")
                nc.vector.tensor_scalar(
                    rte[:], h_q, t_col, None,
                    op0=ALU.max, op1=ALU.add,
                    accum_out=A1[:, q : q + 1],
                )
                sqe = sqt.tile([P, D], F16, tag="sq")
                nc.scalar.activation(
                    sqe[:], rte[:], ACTF.Square,
                    scale=1.0, bias=tb[:, q : q + 1],
                    accum_out=S2[:, q : q + 1],
                )
            if "fixed_t" in ablate:
                return
            # all-DVE 2-step-Newton update, clamped to [-2, s1/k]
            tm = smp.tile([P, W], F32, tag="tm")
            nc.vector.tensor_scalar_mul(tm[:], t16f[:], -1024.0)
            s1 = smp.tile([P, W], F32, tag="s1i")
            nc.vector.tensor_tensor(s1[:], A1[:], tm[:], op=ALU.add)
            s1g = smp.tile([P, W], F32, tag="s1g")
            nc.vector.tensor_scalar_max(s1g[:], s1[:], 1e-6)
            g1 = smp.tile([P, W], F32, tag="g1")
            nc.vector.tensor_scalar(g1[:], S2[:], -1.0, None, op0=ALU.add)
            rp = smp.tile([P, W], F32, tag="rp")
            nc.vector.reciprocal(rp[:], s1g[:])
            a_ = smp.tile([P, W], F32, tag="a_")
            nc.vector.tensor_tensor(a_[:], g1[:], rp[:], op=ALU.mult)
            th1 = smp.tile([P, W], F32, tag="th1")
            nc.vector.tensor_scalar_mul(th1[:], a_[:], 0.5)
            e = smp.tile([P, W], F32, tag="e")
            nc.vector.tensor_tensor(e[:], K[:], th1[:], op=ALU.mult)
            c_ = smp.tile([P, W], F32, tag="c_")
            nc.vector.tensor_tensor(c_[:], e[:], s1g[:], op=ALU.subtract)
            # qv = th1*(e - 2 s1) + g = th1*(c_ - s1) + g
            c2 = smp.tile([P, W], F32, tag="c2")
            nc.vector.tensor_tensor(c2[:], c_[:], s1g[:], op=ALU.subtract)
            u_ = smp.tile([P, W], F32, tag="u_")
            nc.vector.tensor_tensor(u_[:], th1[:], c2[:], op=ALU.mult)
            qv = smp.tile([P, W], F32, tag="qv")
            nc.vector.tensor_tensor(qv[:], u_[:], g1[:], op=ALU.add)
            # qp = 2*(e - s1) = 2*c_
            qp = smp.tile([P, W], F32, tag="qp")
            nc.vector.tensor_scalar_mul(qp[:], c_[:], 2.0)
            rq = smp.tile([P, W], F32, tag="rq")
            nc.vector.reciprocal(rq[:], qp[:])
            d_ = smp.tile([P, W], F32, tag="d_")
            nc.vector.tensor_tensor(d_[:], qv[:], rq[:], op=ALU.mult)
            th2 = smp.tile([P, W], F32, tag="th2")
            nc.vector.tensor_tensor(th2[:], th1[:], d_[:], op=ALU.subtract)
            kg = smp.tile([P, W], F32, tag="kg")
            nc.vector.tensor_scalar_max(kg[:], K[:], 1.0)
            kinv = smp.tile([P, W], F32, tag="kinv")
            nc.vector.reciprocal(kinv[:], kg[:])
            thv = smp.tile([P, W], F32, tag="thv")
            nc.vector.tensor_tensor(thv[:], s1g[:], kinv[:], op=ALU.mult)
            thc = smp.tile([P, W], F32, tag="thc")
            nc.vector.tensor_tensor(thc[:], th2[:], thv[:], op=ALU.min)
            thc2 = smp.tile([P, W], F32, tag="thc2")
            nc.vector.tensor_scalar_max(thc2[:], thc[:], -2.0)
            t_new = smp.tile([P, W], F32, tag="tn")
            nc.vector.tensor_tensor(t_new[:], ps["t"][:], thc2[:], op=ALU.add)
            ps["t"] = t_new

        def stage_final(ps):
            t16f, tb = q16(ps)
            for gi, g in enumerate(ps["pair"]):
                g_row0 = g * G * P
                for b in range(nb):
                    yt = yp.tile([P, B, D], F32, tag="y")
                    for jj in range(B):
                        q = gi * G + b * B + jj
                        h_q = hview(ps, q)
                        rte = rt.tile([P, D], F16, tag="r")
                        nc.vector.tensor_scalar(
                            rte[:], h_q, t16f[:, q : q + 1], None, op0=ALU.max
                        )
                        nc.scalar.activation(
                            yt[:, jj, :], rte[:], ACTF.Square,
                            scale=1.0, bias=tb[:, q : q + 1],
                        )
                    r0 = g_row0 + b * B * P
                    if "no_dma_out" not in ablate:
                        nc.sync.dma_start(
                            y[r0 : r0 + B * P, :].rearrange(
                                "(a p) m -> p a m", p=P
                            ),
                            yt[:],
                        )

        for rep in range(reps):
            if rep == 1:
                src["t"] = y
            for p0 in range(0, n_groups, 2):
                pair = [g for g in (p0, p0 + 1) if g < n_groups]
                ps = stage_load_cast_seed(pair)
                for it in range(2):
                    stage_iter(ps, it)
                stage_final(ps)

    nc.compile()
    return nc


_PROGRAM = None
_PROGRAM_ROWS = None


def _get_program(rows_per_core):
    global _PROGRAM, _PROGRAM_ROWS
    if _PROGRAM is None or _PROGRAM_ROWS != rows_per_core:
        _PROGRAM = build_program(rows_per_core)
        _PROGRAM_ROWS = rows_per_core
    return _PROGRAM


def run_sharded(flat_x, trace=False):
    from concourse.bass_utils import run_bass_kernel_spmd

    n_rows = flat_x.shape[0]
    rows_per = n_rows // N_CORES
    assert rows_per * N_CORES == n_rows
    nc = _get_program(rows_per)
    in_maps = [
        {"x": np.ascontiguousarray(flat_x[i * rows_per : (i + 1) * rows_per])}
        for i in range(N_CORES)
    ]
    res = run_bass_kernel_spmd(nc, in_maps, list(range(N_CORES)), trace=trace)
    y = np.concatenate([res.results[i]["y"] for i in range(N_CORES)], axis=0)
    return y, res


def kernel(x):
    x = np.ascontiguousarray(np.asarray(x), dtype=np.float32)
    orig_shape = x.shape
    flat = x.reshape(-1, D)
    y, _ = run_sharded(flat)
    return y.reshape(orig_shape)


# revision 7
# speedup vs baseline: 12.2486x; 1.6813x over previous
"""Trainium2 Bass kernel v3: entmax-1.5 along last dim of x[8,16,1024,1024] f32.

Row-parallel over 8 NeuronCores. Validated in numpy against the float64
sort reference on 32k real rows: max |y-yref| = 2.55e-3 (gate 2e-2).

Algorithm per row (d=1024), h = f16(x/2), tau in h-space:
  seed: closed-form k=8 threshold over the 8 chunk maxima of h (ACT Sqrt).
  2 support iterations with threshold quantized to f16 and used
  consistently (off-support terms cancel exactly):
    k  = sum(h > t16)                  DVE is_gt + accum
    r  = max(h, t16), A1 = sum r       DVE max + accum
    s2 = sum (r - t16)^2               ACT Square(r, bias=-t16) + accum
    s1 = A1 - 1024 t16; theta = 2-step-Newton root of
         k th^2 - 2 s1 th + (s2-1), clamped to [-2, s1/k]  (all-DVE chain)
  final: r3 = max(h, t16); y = Square(r3 - t16) -> f32.

HW lessons encoded here (all measured on these cores):
  - GPSIMD streaming is ~20x slower than modeled -> Pool unused.
  - 4-dim DMA access patterns (512B descriptors) degrade badly with 8 cores
    running concurrently -> v1-style "(a p) m -> p a m" 4KB descriptors.
  - accum_out ops run ~1us (not 327ns); ACT Square with bias-AP+accum
    ~2.6us; plain streaming ops are fast.
  - Cross-engine dependency hops in the t-update chains are expensive ->
    chains are all-DVE (2-step Newton instead of sqrt; ACT Sqrt only in
    the per-group seed chain), and the two groups of an emission pair
    share one [P, 2G] chain.
"""

import sys

sys.path.insert(0, "/opt/trn_rl_repo")
sys.path.insert(0, "/opt/trn_rl_repo/concourse")

from contextlib import ExitStack

import numpy as np

D = 1024
P = 128
N_CORES = 8


def build_program(n_rows, group_tiles=16, dma_batch=4, debug=False, reps=1,
                  xp_bufs=4, hp_mult=2, tr_bufs=4, yp_bufs=2, smp_bufs=3,
                  ablate=()):
    import concourse.bacc as bacc
    import concourse.tile as tile
    from concourse import mybir

    F32 = mybir.dt.float32
    F16 = mybir.dt.float16
    ALU = mybir.AluOpType
    ACTF = mybir.ActivationFunctionType
    AX = mybir.AxisListType

    G = group_tiles
    B = dma_batch
    nb = G // B
    T = n_rows // P
    assert n_rows % P == 0 and T % G == 0 and G % B == 0
    n_groups = T // G

    nc = bacc.Bacc(
        "TRN2", target_bir_lowering=False, debug=debug, enable_asserts=False
    )
    x = nc.dram_tensor("x", [n_rows, D], F32, kind="ExternalInput").ap()
    y = nc.dram_tensor("y", [n_rows, D], F32, kind="ExternalOutput").ap()

    with tile.TileContext(nc) as tc, ExitStack() as ctx:
        xp = ctx.enter_context(tc.tile_pool(name="xp", bufs=xp_bufs))
        hp = ctx.enter_context(tc.tile_pool(name="hp", bufs=hp_mult * nb))
        c8p = ctx.enter_context(tc.tile_pool(name="c8p", bufs=2))
        rt = ctx.enter_context(tc.tile_pool(name="rt", bufs=tr_bufs))
        kt = ctx.enter_context(tc.tile_pool(name="kt", bufs=tr_bufs))
        sqt = ctx.enter_context(tc.tile_pool(name="sqt", bufs=tr_bufs))
        yp = ctx.enter_context(tc.tile_pool(name="yp", bufs=yp_bufs))
        smp = ctx.enter_context(tc.tile_pool(name="smp", bufs=smp_bufs))

        state = {}
        src = {"t": x}

        # pair-level state: W = columns of the merged chain (2G normally)
        def hview(pair_state, q):
            """flat [P, 1024] f16 view of pair-tile q (0..W-1)."""
            hb = pair_state["hb"][q // B]
            return hb[:, q % B, :]

        def stage_load_cast_seed(pair):
            """Load, cast, chunk-max and seed for ALL groups of the pair;
            one merged [P, W] chain (W = 16*len(pair))."""
            W = G * len(pair)
            ps = {"hb": [], "W": W, "pair": pair}
            c8 = c8p.tile([P, W, 8], F32, tag="c8")
            for gi, g in enumerate(pair):
                g_row0 = g * G * P
                for b in range(nb):
                    xt = xp.tile([P, B, D], F32, tag="x")
                    r0 = g_row0 + b * B * P
                    if "no_dma_in" not in ablate:
                        nc.sync.dma_start(
                            xt[:],
                            src["t"][r0 : r0 + B * P, :].rearrange(
                                "(a p) m -> p a m", p=P
                            ),
                        )
                    else:
                        nc.vector.memset(xt[:, 0:1, 0:8], 0.5)
                    ht = hp.tile([P, B, D], F16, tag="h")
                    nc.vector.tensor_scalar(
                        ht[:], xt[:], 0.5, None, op0=ALU.mult
                    )
                    ps["hb"].append(ht)
                    for jj in range(B):
                        q = gi * G + b * B + jj
                        nc.vector.tensor_reduce(
                            c8[:, q, :],
                            ht[:, jj, :].rearrange("p (c e) -> p c e", e=128),
                            axis=AX.X,
                            op=ALU.max,
                        )

            # merged seed chain (k=8 closed form, ACT Sqrt for the root)
            m = smp.tile([P, W], F32, tag="m")
            nc.vector.tensor_reduce(m[:], c8[:], axis=AX.X, op=ALU.max)
            S = smp.tile([P, W], F32, tag="S")
            nc.vector.tensor_reduce(S[:], c8[:], axis=AX.X, op=ALU.add)
            c8sq = c8p.tile([P, W, 8], F32, tag="c8sq")
            nc.vector.tensor_tensor(c8sq[:], c8[:], c8[:], op=ALU.mult)
            Q = smp.tile([P, W], F32, tag="Q")
            nc.vector.tensor_reduce(Q[:], c8sq[:], axis=AX.X, op=ALU.add)
            mm = smp.tile([P, W], F32, tag="mm")
            nc.vector.tensor_scalar_mul(mm[:], m[:], -8.0)
            s1 = smp.tile([P, W], F32, tag="s1")
            nc.vector.tensor_tensor(s1[:], S[:], mm[:], op=ALU.add)
            mS = smp.tile([P, W], F32, tag="mS")
            nc.vector.tensor_tensor(mS[:], m[:], S[:], op=ALU.mult)
            m2 = smp.tile([P, W], F32, tag="m2")
            nc.vector.tensor_tensor(m2[:], m[:], m[:], op=ALU.mult)
            a1 = smp.tile([P, W], F32, tag="a1")
            nc.vector.tensor_scalar_mul(a1[:], mS[:], -2.0)
            a2 = smp.tile([P, W], F32, tag="a2")
            nc.vector.tensor_scalar_mul(a2[:], m2[:], 8.0)
            s2a = smp.tile([P, W], F32, tag="s2a")
            nc.vector.tensor_tensor(s2a[:], Q[:], a1[:], op=ALU.add)
            s2 = smp.tile([P, W], F32, tag="s2")
            nc.vector.tensor_tensor(s2[:], s2a[:], a2[:], op=ALU.add)
            q_ = smp.tile([P, W], F32, tag="q")
            nc.vector.tensor_tensor(q_[:], s1[:], s1[:], op=ALU.mult)
            b1 = smp.tile([P, W], F32, tag="b1")
            nc.vector.tensor_scalar(
                b1[:], s2[:], -8.0, 8.0, op0=ALU.mult, op1=ALU.add
            )
            d0 = smp.tile([P, W], F32, tag="d0")
            nc.vector.tensor_tensor(d0[:], q_[:], b1[:], op=ALU.add)
            dn = smp.tile([P, W], F32, tag="dn")
            nc.vector.tensor_scalar_max(dn[:], d0[:], 1e-30)
            root = smp.tile([P, W], F32, tag="root")
            nc.scalar.activation(root[:], dn[:], ACTF.Sqrt)
            num = smp.tile([P, W], F32, tag="num")
            nc.vector.tensor_tensor(num[:], s1[:], root[:], op=ALU.subtract)
            th = smp.tile([P, W], F32, tag="th")
            nc.vector.tensor_scalar_mul(th[:], num[:], 0.125)
            t0 = smp.tile([P, W], F32, tag="t0")
            nc.vector.tensor_tensor(t0[:], m[:], th[:], op=ALU.add)
            ps["t"] = t0
            return ps

        def q16(ps):
            W = ps["W"]
            if "fixed_t" in ablate:
                t16f = smp.tile([P, W], F32, tag="t16f")
                nc.vector.memset(t16f[:], 1.25)
                tb = smp.tile([P, W], F32, tag="tb")
                nc.vector.memset(tb[:], -1.25)
                return t16f, tb
            t16h = smp.tile([P, W], F16, tag="t16h")
            nc.vector.tensor_scalar_mul(t16h[:], ps["t"][:], 1.0)
            t16f = smp.tile([P, W], F32, tag="t16f")
            nc.vector.tensor_scalar_mul(t16f[:], t16h[:], 1.0)
            tb = smp.tile([P, W], F32, tag="tb")
            nc.vector.tensor_scalar_mul(tb[:], t16f[:], -1.0)
            return t16f, tb

        def stage_iter(ps, it):
            W = ps["W"]
            t16f, tb = q16(ps)
            if it == 0:
                K = smp.tile([P, W], F32, tag="K")
                ps["K"] = K
            else:
                K = ps["K"]  # reuse iter-1 support count (validated 4.5e-3)
            A1 = smp.tile([P, W], F32, tag="A1")
            S2 = smp.tile([P, W], F32, tag="S2")
            for q in range(W):
                h_q = hview(ps, q)
                t_col = t16f[:, q : q + 1]
                if it == 0:
                    kct = kt.tile([P, D], F16, tag="k")
                    nc.vector.tensor_scalar(
                        kct[:], h_q, t_col, None,
                        op0=ALU.is_gt, op1=ALU.add,
                        accum_out=K[:, q : q + 1],
                    )
                rte = rt.tile([P, D], F16, tag="r")
                nc.vector.tensor_scalar(
                    rte[:], h_q, t_col, None,
                    op0=ALU.max, op1=ALU.add,
                    accum_out=A1[:, q : q + 1],
                )
                sqe = sqt.tile([P, D], F16, tag="sq")
                nc.scalar.activation(
                    sqe[:], rte[:], ACTF.Square,
                    scale=1.0, bias=tb[:, q : q + 1],
                    accum_out=S2[:, q : q + 1],
                )
            if "fixed_t" in ablate:
                return
            # all-DVE 2-step-Newton update, clamped to [-2, s1/k]
            tm = smp.tile([P, W], F32, tag="tm")
            nc.vector.tensor_scalar_mul(tm[:], t16f[:], -1024.0)
            s1 = smp.tile([P, W], F32, tag="s1i")
            nc.vector.tensor_tensor(s1[:], A1[:], tm[:], op=ALU.add)
            s1g = smp.tile([P, W], F32, tag="s1g")
            nc.vector.tensor_scalar_max(s1g[:], s1[:], 1e-6)
            g1 = smp.tile([P, W], F32, tag="g1")
            nc.vector.tensor_scalar(g1[:], S2[:], -1.0, None, op0=ALU.add)
            rp = smp.tile([P, W], F32, tag="rp")
            nc.vector.reciprocal(rp[:], s1g[:])
            a_ = smp.tile([P, W], F32, tag="a_")
            nc.vector.tensor_tensor(a_[:], g1[:], rp[:], op=ALU.mult)
            th1 = smp.tile([P, W], F32, tag="th1")
            nc.vector.tensor_scalar_mul(th1[:], a_[:], 0.5)
            e = smp.tile([P, W], F32, tag="e")
            nc.vector.tensor_tensor(e[:], K[:], th1[:], op=ALU.mult)
            c_ = smp.tile([P, W], F32, tag="c_")
            nc.vector.tensor_tensor(c_[:], e[:], s1g[:], op=ALU.subtract)
            # qv = th1*(e - 2 s1) + g = th1*(c_ - s1) + g
            c2 = smp.tile([P, W], F32, tag="c2")
            nc.vector.tensor_tensor(c2[:], c_[:], s1g[:], op=ALU.subtract)
            u_ = smp.tile([P, W], F32, tag="u_")
            nc.vector.tensor_tensor(u_[:], th1[:], c2[:], op=ALU.mult)
            qv = smp.tile([P, W], F32, tag="qv")
            nc.vector.tensor_tensor(qv[:], u_[:], g1[:], op=ALU.add)
            # qp = 2*(e - s1) = 2*c_
            qp = smp.tile([P, W], F32, tag="qp")
            nc.vector.tensor_scalar_mul(qp[:], c_[:], 2.0)
            rq = smp.tile([P, W], F32, tag="rq")
            nc.vector.reciprocal(rq[:], qp[:])
            d_ = smp.tile([P, W], F32, tag="d_")
            nc.vector.tensor_tensor(d_[:], qv[:], rq[:], op=ALU.mult)
            th2 = smp.tile([P, W], F32, tag="th2")
            nc.vector.tensor_tensor(th2[:], th1[:], d_[:], op=ALU.subtract)
            kg = smp.tile([P, W], F32, tag="kg")
            nc.vector.tensor_scalar_max(kg[:], K[:], 1.0)
            kinv = smp.tile([P, W], F32, tag="kinv")
            nc.vector.reciprocal(kinv[:], kg[:])
            thv = smp.tile([P, W], F32, tag="thv")
            nc.vector.tensor_tensor(thv[:], s1g[:], kinv[:], op=ALU.mult)
            thc = smp.tile([P, W], F32, tag="thc")
            nc.vector.tensor_tensor(thc[:], th2[:], thv[:], op=ALU.min)
            thc2 = smp.tile([P, W], F32, tag="thc2")
            nc.vector.tensor_scalar_max(thc2[:], thc[:], -2.0)
            t_new = smp.tile([P, W], F32, tag="tn")
            nc.vector.tensor_tensor(t_new[:], ps["t"][:], thc2[:], op=ALU.add)
            ps["t"] = t_new

        def stage_final(ps):
            t16f, tb = q16(ps)
            for gi, g in enumerate(ps["pair"]):
                g_row0 = g * G * P
                for b in range(nb):
                    yt = yp.tile([P, B, D], F32, tag="y")
                    for jj in range(B):
                        q = gi * G + b * B + jj
                        h_q = hview(ps, q)
                        rte = rt.tile([P, D], F16, tag="r")
                        nc.vector.tensor_scalar(
                            rte[:], h_q, t16f[:, q : q + 1], None, op0=ALU.max
                        )
                        nc.scalar.activation(
                            yt[:, jj, :], rte[:], ACTF.Square,
                            scale=1.0, bias=tb[:, q : q + 1],
                        )
                    r0 = g_row0 + b * B * P
                    if "no_dma_out" not in ablate:
                        nc.sync.dma_start(
                            y[r0 : r0 + B * P, :].rearrange(
                                "(a p) m -> p a m", p=P
                            ),
                            yt[:],
                        )

        for rep in range(reps):
            if rep == 1:
                src["t"] = y
            for p0 in range(0, n_groups, 2):
                pair = [g for g in (p0, p0 + 1) if g < n_groups]
                ps = stage_load_cast_seed(pair)
                for it in range(2):
                    stage_iter(ps, it)
                stage_final(ps)

    nc.compile()
    return nc


_PROGRAM = None
_PROGRAM_ROWS = None


def _get_program(rows_per_core):
    global _PROGRAM, _PROGRAM_ROWS
    if _PROGRAM is None or _PROGRAM_ROWS != rows_per_core:
        _PROGRAM = build_program(rows_per_core)
        _PROGRAM_ROWS = rows_per_core
    return _PROGRAM


def run_sharded(flat_x, trace=False):
    from concourse.bass_utils import run_bass_kernel_spmd

    n_rows = flat_x.shape[0]
    rows_per = n_rows // N_CORES
    assert rows_per * N_CORES == n_rows
    nc = _get_program(rows_per)
    in_maps = [
        {"x": np.ascontiguousarray(flat_x[i * rows_per : (i + 1) * rows_per])}
        for i in range(N_CORES)
    ]
    res = run_bass_kernel_spmd(nc, in_maps, list(range(N_CORES)), trace=trace)
    y = np.concatenate([res.results[i]["y"] for i in range(N_CORES)], axis=0)
    return y, res


def kernel(x):
    x = np.ascontiguousarray(np.asarray(x), dtype=np.float32)
    orig_shape = x.shape
    flat = x.reshape(-1, D)
    y, _ = run_sharded(flat)
    return y.reshape(orig_shape)
